# revision 10
# baseline (speedup 1.0000x reference)
"""2-layer GCN (GCNConv x2 + log_softmax) on 8 Trainium2 NeuronCores.

Sharding: destination-node rows are split across the 8 cores (S=12544 nodes
each, incl. 352 zero phantom pad nodes); W1/W2 replicated. Per layer the
scaled feature table g = deg^-1/2 * (h @ W) is all-gathered into every core
as a feature-transposed SBUF table [128, S] (partition 16*g+f = feature f of
source shard g). The per-edge gather runs on GPSIMD ap_gather (8 Q7 cores,
one per source shard). Segment sums use an exact-length-class edge layout
(host-sorted) + DVE strided reduces, then an ap_gather un-permute back to
node order and a PE matmul combine across the 8 source groups.

Perf structure: the table is appended to classout in one SBUF tile [128,
T+S], so degree-1-in-group dests skip the edge stream entirely (their
aggregate IS one table column; the un-permute fetches it directly). The
un-permute is sliced per 512-column post tile so the PE/DVE post phase
overlaps the Pool gather instead of serializing after it.
"""
import numpy as np

NC = 8            # cores
NG = 8            # source groups (= shards)
S = 12544         # nodes per shard
V = NC * S        # padded node count
N_REAL = 100000
F = 16            # hidden dim
C8 = 8            # padded class count
X = 512           # input dim
TILE = 512        # post-phase column tile
GATHER_SLICE = 8192
CHUNKS = [(0, 4096), (4096, 8192), (8192, 12544)]  # collective col chunks
ALIGN = 32        # stream cut alignment (idx offsets must be 4B aligned)


# --------------------------------------------------------------- tile patch
def _install_tile_patch():
    """The Tile tail drain accumulates more sem waits than this compiler
    allows on one CTRL instruction; spread them over SP nops (1 wait each)."""
    import concourse.tile as tile
    import concourse.mybir as mybir
    from concourse.vector_clock import ScopedClock
    if getattr(tile.TileContext, "_drain_patch", False):
        return
    _MAX_WAITS = 1

    def _patched(self, tick_clock, wait_clock):
        nc = self.nc
        nops = [nc.sync.nop(nofuse=True) for _ in range(40)]
        drain_inst = nc.sync.drain()
        wait_clock.add_sem_waits(
            drain_inst.ins, ScopedClock({None: tick_clock.global_clock})
        )
        si = drain_inst.ins.sync_info
        if si is not None and si.on_wait and len(si.on_wait) > _MAX_WAITS:
            waits = list(si.on_wait)
            si.on_wait.clear()
            chunks = [waits[i:i + _MAX_WAITS] for i in range(0, len(waits), _MAX_WAITS)]
            si.on_wait.extend(chunks[-1])
            rest = chunks[:-1]
            assert len(rest) <= len(nops), f"too many wait chunks: {len(rest)}"
            for nop, chunk in zip(nops, rest):
                nsi = nop.ins.sync_info
                if nsi is None:
                    nop.ins.sync_info = mybir.SyncInfo(on_wait=list(chunk), on_update=[])
                else:
                    nsi.on_wait.extend(chunk)
        nc.all_engine_barrier()
        assert self.sems is not None
        popped = nc._tile_sem_poison_stack.pop()
        assert popped is self._sem_poison
        nc.clear_and_free_semaphores(list(self.sems.allocated().values()))
        nc.all_engine_barrier()

    tile.TileContext._drain_and_barrier = _patched
    tile.TileContext._drain_patch = True


# ---------------------------------------------------------------- host prep
def preprocess(edge_index):
    row = np.asarray(edge_index[0], dtype=np.int64)
    col = np.asarray(edge_index[1], dtype=np.int64)
    deg_full = (np.bincount(col, minlength=V) + 1).astype(np.int32)

    core_of = (col // S).astype(np.int32)
    per_core = []
    cnt_all = np.zeros((NC, NG, S), dtype=np.int64)
    for k in range(NC):
        m = core_of == k
        r, c = row[m], col[m]
        g = (r // S).astype(np.int64)
        sloc = (r - g * S).astype(np.int64)
        dloc = (c - k * S).astype(np.int64)
        cnt = np.bincount(g * S + dloc, minlength=NG * S).reshape(NG, S)
        cnt_all[k] = cnt
        per_core.append((g, sloc, dloc, cnt))

    cmax = int(cnt_all.max())
    ncls = np.zeros((NC, NG, cmax + 1), dtype=np.int64)
    for k in range(NC):
        for g in range(NG):
            ncls[k, g] = np.bincount(cnt_all[k, g], minlength=cmax + 1)
    n_glob = ncls.max(axis=(0, 1))
    classes = [c for c in range(2, cmax + 1) if n_glob[c] > 0]  # c==1 via unperm
    n_pad = {}
    for c in classes:
        n = int(n_glob[c])
        t = 0
        while ((n + t) * c) % ALIGN != 0:
            t += 1
        n_pad[c] = n + t
    stream_start, pos_start = {}, {}
    soff, poff = 0, 1          # output position 0 reserved as zero
    for c in classes:
        stream_start[c] = soff
        pos_start[c] = poff
        soff += n_pad[c] * c
        poff += n_pad[c]
    L, T = soff, poff
    assert L % ALIGN == 0 and T < 32768

    cuts = [0]
    for c in classes:
        step = c
        while step % ALIGN != 0:
            step += c
        rs, re = stream_start[c], stream_start[c] + n_pad[c] * c
        cur = cuts[-1]
        while True:
            nxt = min(cur + GATHER_SLICE, re)
            if nxt >= re:
                if re > cur:
                    cuts.append(re)
                break
            snapped = rs + ((nxt - rs) // step) * step
            if snapped <= cur:
                snapped = min(cur + step, re)
            cuts.append(snapped)
            cur = snapped
    assert cuts[-1] == L
    slices = list(zip(cuts[:-1], cuts[1:]))
    max_slice = max(b - a for a, b in slices)

    slice_ops = []
    for a, b in slices:
        ops = []
        for c in classes:
            rs, re = stream_start[c], stream_start[c] + n_pad[c] * c
            lo, hi = max(rs, a), min(re, b)
            if lo >= hi:
                continue
            ops.append((lo - a, c, (hi - lo) // c, pos_start[c] + (lo - rs) // c))
        slice_ops.append(ops)

    inputs = []
    for k in range(NC):
        g, sloc, dloc, cnt = per_core[k]
        idx_streams = np.zeros((NG, L), dtype=np.int16)
        pos_arr = np.zeros((NG, S), dtype=np.int16)
        for gg in range(NG):
            m = g == gg
            sl, dl = sloc[m], dloc[m]
            cc = cnt[gg, dl]
            sl_raw, cc_raw = sl, cc
            order = np.lexsort((sl, dl, cc))
            sl, cc = sl[order], cc[order]
            bnd = np.searchsorted(cc, np.arange(1, cmax + 2))
            for c in classes:
                lo, hi = bnd[c - 1], bnd[c]
                if hi > lo:
                    idx_streams[gg, stream_start[c]:stream_start[c] + (hi - lo)] = sl[lo:hi]
            arr = cnt[gg]
            src1 = np.zeros(S, dtype=np.int64)
            e1 = cc_raw == 1
            src1[dl[e1]] = sl_raw[e1]
            sorted_d = np.argsort(arr, kind="stable")
            arr_s = arr[sorted_d]
            first = np.searchsorted(arr_s, arr_s)
            rank = np.arange(S) - first
            ps = np.zeros(S, dtype=np.int64)
            starts = np.array([pos_start[int(c)] if c > 1 else 0 for c in arr_s])
            ps[sorted_d] = np.where(arr_s > 1, starts + rank, 0)
            ps = np.where(arr == 1, T + src1, ps)
            pos_arr[gg] = ps.astype(np.int16)

        def pack16(mat, width):
            out = np.zeros((128, width // 16), np.int16)
            for gg in range(NG):
                out[16 * gg:16 * gg + 16, :] = mat[gg].reshape(width // 16, 16).T
            return out

        inputs.append({
            "idx": pack16(idx_streams, L),
            "unperm": pack16(pos_arr, S),
            "deg": deg_full[k * S:(k + 1) * S].reshape(1, S),
        })
    meta = dict(L=L, T=T, slices=slices, slice_ops=slice_ops, max_slice=max_slice)
    return inputs, meta


# ---------------------------------------------------------------- kernel
def build_kernel(meta, reps=1):
    import concourse.bass as bass
    import concourse.mybir as mybir
    from concourse import bacc
    from concourse.tile import TileContext
    _install_tile_patch()
    AF = mybir.ActivationFunctionType
    DT = mybir.dt
    L, T = meta["L"], meta["T"]
    slices, slice_ops = meta["slices"], meta["slice_ops"]
    max_slice = meta["max_slice"]

    nc = bacc.Bacc(None, target_bir_lowering=False, num_devices=NC)
    f32 = DT.float32

    xt_d = nc.dram_tensor("xt", [X, S], f32, kind="ExternalInput")
    deg_d = nc.dram_tensor("deg", [1, S], DT.int32, kind="ExternalInput")
    idx_d = nc.dram_tensor("idx", [128, L // 16], DT.int16, kind="ExternalInput")
    unp_d = nc.dram_tensor("unperm", [128, S // 16], DT.int16, kind="ExternalInput")
    w1_d = nc.dram_tensor("W1", [X, F], f32, kind="ExternalInput")
    b1_d = nc.dram_tensor("b1", [F, 1], f32, kind="ExternalInput")
    w2_d = nc.dram_tensor("W2", [F, C8], f32, kind="ExternalInput")
    b2_d = nc.dram_tensor("b2", [C8, 1], f32, kind="ExternalInput")
    pcomb_d = nc.dram_tensor("pcomb", [128, F], f32, kind="ExternalInput")
    ones7_d = nc.dram_tensor("ones7", [C8, 1], f32, kind="ExternalInput")
    ones18_d = nc.dram_tensor("ones18", [1, C8], f32, kind="ExternalInput")
    lg_d = nc.dram_tensor("lg_t", [C8, S], f32, kind="ExternalOutput")
    ls_d = nc.dram_tensor("ls_t", [C8, S], f32, kind="ExternalOutput")

    def widths():
        off = 0
        while off < S:
            w = min(TILE, S - off)
            yield off, w
            off += w

    with TileContext(nc) as tc:
        with tc.tile_pool(name="dram", bufs=1, space="DRAM") as dram, \
             tc.tile_pool(name="const", bufs=1) as constp:
            idx_t = constp.tile([128, L // 16], DT.int16)
            nc.sync.dma_start(out=idx_t[:], in_=idx_d[:])
            unp_t = constp.tile([128, S // 16], DT.int16)
            nc.sync.dma_start(out=unp_t[:], in_=unp_d[:])
            w1_t = constp.tile([128, 4 * F], f32)
            for kc in range(4):
                nc.sync.dma_start(out=w1_t[:, kc * F:(kc + 1) * F],
                                  in_=w1_d[kc * 128:(kc + 1) * 128, :])
            w2_t = constp.tile([F, C8], f32)
            nc.sync.dma_start(out=w2_t[:], in_=w2_d[:])
            b1_t = constp.tile([F, 1], f32)
            nc.sync.dma_start(out=b1_t[:], in_=b1_d[:])
            b2_t = constp.tile([C8, 1], f32)
            nc.sync.dma_start(out=b2_t[:], in_=b2_d[:])
            pcomb_t = constp.tile([128, F], f32)
            nc.sync.dma_start(out=pcomb_t[:], in_=pcomb_d[:])
            ones7_t = constp.tile([C8, 1], f32)
            nc.sync.dma_start(out=ones7_t[:], in_=ones7_d[:])
            ones18_t = constp.tile([1, C8], f32)
            nc.sync.dma_start(out=ones18_t[:], in_=ones18_d[:])
            zero8_t = constp.tile([C8, TILE], f32)
            nc.gpsimd.memset(zero8_t[:], 0.0)

          # (reps>1 repeats the whole pipeline for HW timing)
            for _rep in range(reps):
                dinv_d = dram.tile([1, S], f32, tag=f"dinv{_rep}")
                cc_in1 = [dram.tile([F, b - a], f32, tag=f"ci1_{_rep}_{j}", name=f"ci1_{_rep}_{j}")
                          for j, (a, b) in enumerate(CHUNKS)]
                cc_out1 = [dram.tile([128, b - a], f32, addr_space="Shared",
                                     tag=f"co1_{_rep}_{j}", name=f"co1_{_rep}_{j}")
                           for j, (a, b) in enumerate(CHUNKS)]
                cc_in2 = [dram.tile([F, b - a], f32, tag=f"ci2_{_rep}_{j}", name=f"ci2_{_rep}_{j}")
                          for j, (a, b) in enumerate(CHUNKS)]
                cc_out2 = [dram.tile([128, b - a], f32, addr_space="Shared",
                                     tag=f"co2_{_rep}_{j}", name=f"co2_{_rep}_{j}")
                           for j, (a, b) in enumerate(CHUNKS)]

                def chunk_of(off):
                    for j, (a, b) in enumerate(CHUNKS):
                        if a <= off < b:
                            return j, off - a
                    raise ValueError(off)
                with tc.tile_pool(name=f"p0_{_rep}", bufs=1) as p0:
                    dg = p0.tile([1, S], DT.int32)
                    nc.sync.dma_start(out=dg[:], in_=deg_d[:])
                    dgf = p0.tile([1, S], f32)
                    nc.vector.tensor_copy(out=dgf[:], in_=dg[:])
                    sq = p0.tile([1, S], f32)
                    nc.scalar.activation(out=sq[:], in_=dgf[:], func=AF.Sqrt)
                    dv = p0.tile([1, S], f32)
                    nc.vector.reciprocal(out=dv[:], in_=sq[:])
                    nc.sync.dma_start(out=dinv_d[:], in_=dv[:])

                with tc.tile_pool(name=f"mmx{_rep}", bufs=4) as mmx, \
                     tc.tile_pool(name=f"mmo{_rep}", bufs=3) as mmo, \
                     tc.tile_pool(name=f"ps1_{_rep}", bufs=3, space="PSUM") as ps1:
                    for off, w in widths():
                        psum = ps1.tile([F, TILE], f32, tag="ps")
                        for kc in range(4):
                            xt_t = mmx.tile([128, TILE], f32, tag="xt")
                            nc.sync.dma_start(
                                out=xt_t[:, :w],
                                in_=xt_d[kc * 128:(kc + 1) * 128, off:off + w])
                            nc.tensor.matmul(
                                out=psum[:, :w],
                                lhsT=w1_t[:, kc * F:(kc + 1) * F],
                                rhs=xt_t[:, :w],
                                start=(kc == 0), stop=(kc == 3))
                        dvr = mmo.tile([F, TILE], f32, tag="dvr")
                        nc.sync.dma_start(
                            out=dvr[:, :w],
                            in_=dinv_d[0:1, off:off + w].to_broadcast([F, w]))
                        g1 = mmo.tile([F, TILE], f32, tag="g1")
                        nc.vector.tensor_tensor(out=g1[:, :w], in0=psum[:, :w],
                                                in1=dvr[:, :w],
                                                op=mybir.AluOpType.mult)
                        j, offl = chunk_of(off)
                        nc.sync.dma_start(out=cc_in1[j][:, offl:offl + w],
                                          in_=g1[:, :w])
                        nc.sync.dma_start(out=cc_in2[j][C8:F, offl:offl + w],
                                          in_=zero8_t[:, :w])

                for layer in (1, 2):
                    cc_in = cc_in1 if layer == 1 else cc_in2
                    cc_out = cc_out1 if layer == 1 else cc_out2
                    for j in range(len(CHUNKS)):
                        nc.gpsimd.collective_compute(
                            "AllGather", mybir.AluOpType.bypass,
                            replica_groups=[list(range(NC))],
                            ins=[cc_in[j][:]], outs=[cc_out[j][:]])

                    with tc.tile_pool(name=f"cls{layer}_{_rep}", bufs=1) as clsp:
                        classout = clsp.tile([128, T + S], f32)
                        nc.gpsimd.memset(classout[:, 0:1], 0.0)
                        with tc.tile_pool(name=f"sl{layer}_{_rep}", bufs=2) as slp:
                            table = classout[:, T:T + S]
                            for j, (a, b) in enumerate(CHUNKS):
                                nc.sync.dma_start(out=classout[:, T + a:T + b],
                                                  in_=cc_out[j][:])
                            for (a, b), ops in zip(slices, slice_ops):
                                ln = b - a
                                sl = slp.tile([128, max_slice], f32, tag="sl")
                                nc.gpsimd.ap_gather(
                                    out_ap=sl[:, :ln].rearrange("p (n d) -> p n d", d=1),
                                    in_ap=table.rearrange("p (n d) -> p n d", d=1),
                                    idxs_ap=idx_t[:, a // 16:b // 16],
                                    channels=128, num_elems=S, d=1, num_idxs=ln)
                                for (loff, c, nseg, pos) in ops:
                                    if c == 1:
                                        nc.vector.tensor_copy(
                                            out=classout[:, pos:pos + nseg],
                                            in_=sl[:, loff:loff + nseg])
                                    else:
                                        nc.vector.tensor_reduce(
                                            out=classout[:, pos:pos + nseg],
                                            in_=sl[:, loff:loff + nseg * c]
                                                .rearrange("p (n c) -> p n c", c=c),
                                            axis=mybir.AxisListType.X,
                                            op=mybir.AluOpType.add)

                        with tc.tile_pool(name=f"al{layer}_{_rep}", bufs=3) as alp:
                            with tc.tile_pool(name=f"po{layer}_{_rep}", bufs=3) as po, \
                                 tc.tile_pool(name=f"pp{layer}_{_rep}", bufs=2,
                                              space="PSUM") as pp:
                                for off, w in widths():
                                    aligned = alp.tile([128, TILE], f32, tag="al")
                                    nc.gpsimd.ap_gather(
                                        out_ap=aligned[:, :w]
                                            .rearrange("p (n d) -> p n d", d=1),
                                        in_ap=classout[:]
                                            .rearrange("p (n d) -> p n d", d=1),
                                        idxs_ap=unp_t[:, off // 16:(off + w) // 16],
                                        channels=128, num_elems=T + S, d=1,
                                        num_idxs=w)
                                    agg = pp.tile([F, TILE], f32, tag="agg")
                                    nc.tensor.matmul(
                                        out=agg[:, :w], lhsT=pcomb_t[:],
                                        rhs=aligned[:, :w],
                                        start=True, stop=True)
                                    j, offl = chunk_of(off)
                                    own = po.tile([F, TILE], f32, tag="own")
                                    nc.sync.dma_start(
                                        out=own[:, :w],
                                        in_=cc_in[j][:, offl:offl + w])
                                    dvr = po.tile([F, TILE], f32, tag="dvr")
                                    nc.sync.dma_start(
                                        out=dvr[:, :w],
                                        in_=dinv_d[0:1, off:off + w].to_broadcast([F, w]))
                                    t0 = po.tile([F, TILE], f32, tag="t0")
                                    nc.vector.tensor_add(out=t0[:, :w], in0=agg[:, :w],
                                                         in1=own[:, :w])
                                    nc.vector.tensor_tensor(out=t0[:, :w], in0=t0[:, :w],
                                                            in1=dvr[:, :w],
                                                            op=mybir.AluOpType.mult)
                                    if layer == 1:
                                        h1 = po.tile([F, TILE], f32, tag="h1")
                                        nc.scalar.activation(out=h1[:, :w], in_=t0[:, :w],
                                                             func=AF.Relu,
                                                             bias=b1_t[:, 0:1])
                                        t2 = pp.tile([C8, TILE], f32, tag="t2")
                                        nc.tensor.matmul(out=t2[:, :w], lhsT=w2_t[:],
                                                         rhs=h1[:, :w],
                                                         start=True, stop=True)
                                        g2 = po.tile([C8, TILE], f32, tag="g2")
                                        nc.vector.tensor_tensor(out=g2[:, :w],
                                                                in0=t2[:, :w],
                                                                in1=dvr[:C8, :w],
                                                                op=mybir.AluOpType.mult)
                                        nc.sync.dma_start(
                                            out=cc_in2[j][:C8, offl:offl + w],
                                            in_=g2[:, :w])
                                    else:
                                        lg = po.tile([C8, TILE], f32, tag="lg")
                                        nc.scalar.activation(out=lg[:, :w],
                                                             in_=t0[:C8, :w],
                                                             func=AF.Identity,
                                                             bias=b2_t[:, 0:1])
                                        nc.sync.dma_start(out=lg_d[:, off:off + w],
                                                          in_=lg[:, :w])
                                        ex = po.tile([C8, TILE], f32, tag="ex")
                                        nc.scalar.activation(out=ex[:, :w],
                                                             in_=lg[:, :w], func=AF.Exp)
                                        sm = pp.tile([1, TILE], f32, tag="sm")
                                        nc.tensor.matmul(out=sm[:, :w], lhsT=ones7_t[:],
                                                         rhs=ex[:, :w],
                                                         start=True, stop=True)
                                        lsm = po.tile([1, TILE], f32, tag="lsm")
                                        nc.scalar.activation(out=lsm[:, :w],
                                                             in_=sm[:, :w], func=AF.Ln)
                                        lsb = pp.tile([C8, TILE], f32, tag="lsb")
                                        nc.tensor.matmul(out=lsb[:, :w],
                                                         lhsT=ones18_t[:],
                                                         rhs=lsm[:, :w],
                                                         start=True, stop=True)
                                        ls = po.tile([C8, TILE], f32, tag="ls")
                                        nc.vector.tensor_tensor(
                                            out=ls[:, :w], in0=lg[:, :w],
                                            in1=lsb[:, :w],
                                            op=mybir.AluOpType.subtract)
                                        nc.sync.dma_start(out=ls_d[:, off:off + w],
                                                          in_=ls[:, :w])
    nc.compile()
    return nc


def make_const_inputs(W1, b1, W2, b2):
    pcomb = np.zeros((128, F), np.float32)
    for g in range(NG):
        for f in range(F):
            pcomb[16 * g + f, f] = 1.0
    ones7 = np.zeros((C8, 1), np.float32); ones7[:7, 0] = 1.0
    ones18 = np.ones((1, C8), np.float32)
    w2p = np.zeros((F, C8), np.float32); w2p[:, :7] = np.asarray(W2, np.float32)
    b2p = np.zeros((C8, 1), np.float32); b2p[:7, 0] = np.asarray(b2, np.float32)
    return {
        "W1": np.asarray(W1, np.float32),
        "b1": np.asarray(b1, np.float32).reshape(F, 1),
        "W2": w2p, "b2": b2p,
        "pcomb": pcomb, "ones7": ones7, "ones18": ones18,
    }


def prepare_all(x, edge_index, W1, b1, W2, b2):
    per_core, meta = preprocess(edge_index)
    consts = make_const_inputs(W1, b1, W2, b2)
    xt = np.zeros((X, V), np.float32)
    xt[:, :N_REAL] = np.asarray(x, np.float32).T
    in_maps = []
    for k in range(NC):
        m = dict(per_core[k])
        m.update(consts)
        m["xt"] = np.ascontiguousarray(xt[:, k * S:(k + 1) * S])
        in_maps.append(m)
    return in_maps, meta


# ---------------------------------------------------------------- runner
class SpmdRunner:
    def __init__(self, nc, n_cores=NC):
        import jax
        import concourse.mybir as mybir
        from concourse import bass2jax
        from jax.sharding import Mesh, PartitionSpec
        from jax.experimental.shard_map import shard_map
        bass2jax.install_neuronx_cc_hook()
        self.jax = jax
        self.n_cores = n_cores
        partition_name = nc.partition_id_tensor.name if nc.partition_id_tensor else None
        in_names, out_names, out_avals, zero_outs = [], [], [], []
        for alloc in nc.m.functions[0].allocations:
            if not isinstance(alloc, mybir.MemoryLocationSet):
                continue
            name = alloc.memorylocations[0].name
            if alloc.kind == "ExternalInput":
                if name != partition_name:
                    in_names.append(name)
            elif alloc.kind == "ExternalOutput":
                shape = tuple(alloc.tensor_shape)
                dtype = mybir.dt.np(alloc.dtype)
                out_names.append(name)
                out_avals.append(jax.core.ShapedArray(shape, dtype))
                zero_outs.append(np.zeros(shape, dtype))
        self.in_names, self.out_names = in_names, out_names
        self.out_avals, self.zero_outs = out_avals, zero_outs
        n_params, n_outs = len(in_names), len(out_names)
        all_in_names = list(in_names) + list(out_names)
        if partition_name is not None:
            all_in_names.append(partition_name)

        def _body(*args):
            operands = list(args)
            if partition_name is not None:
                operands.append(bass2jax.partition_id_tensor())
            outs = bass2jax._bass_exec_p.bind(
                *operands,
                out_avals=tuple(out_avals),
                in_names=tuple(all_in_names),
                out_names=tuple(out_names),
                lowering_input_output_aliases=(),
                sim_require_finite=False,
                sim_require_nnan=False,
                nc=nc,
            )
            return tuple(outs)

        devices = jax.devices()[:n_cores]
        self.mesh = Mesh(np.asarray(devices), ("core",))
        in_specs = (PartitionSpec("core"),) * (n_params + n_outs)
        out_specs = (PartitionSpec("core"),) * n_outs
        self.fn = jax.jit(
            shard_map(_body, mesh=self.mesh, in_specs=in_specs,
                      out_specs=out_specs, check_rep=False),
            keep_unused=True,
        )

    def _concat(self, in_maps):
        n = self.n_cores
        per_core = [[np.asarray(m[name]) for name in self.in_names] for m in in_maps]
        concat_in = [np.concatenate([per_core[c][i] for c in range(n)], axis=0)
                     for i in range(len(self.in_names))]
        concat_zeros = [np.zeros((n * z.shape[0], *z.shape[1:]), z.dtype)
                        for z in self.zero_outs]
        return concat_in + concat_zeros

    def __call__(self, in_maps):
        jax = self.jax
        out = self.fn(*self._concat(in_maps))
        jax.block_until_ready(out)
        n = self.n_cores
        return [
            {name: np.asarray(out[i]).reshape(n, *self.out_avals[i].shape)[c]
             for i, name in enumerate(self.out_names)}
            for c in range(n)
        ]

    def time_it(self, in_maps, reps=5):
        import time
        jax = self.jax
        from jax.sharding import NamedSharding, PartitionSpec
        sh = NamedSharding(self.mesh, PartitionSpec("core"))
        args = [jax.device_put(a, sh) for a in self._concat(in_maps)]
        out = self.fn(*args); jax.block_until_ready(out)
        ts = []
        for _ in range(reps):
            t0 = time.perf_counter()
            out = self.fn(*args)
            jax.block_until_ready(out)
            ts.append(time.perf_counter() - t0)
        return min(ts), ts


def postprocess(results):
    lg = np.concatenate([r["lg_t"].T for r in results], axis=0)[:N_REAL, :7]
    ls = np.concatenate([r["ls_t"].T for r in results], axis=0)[:N_REAL, :7]
    return lg, ls


_CACHE = {}


def kernel(x, edge_index, W1, b1, W2, b2):
    in_maps, meta = prepare_all(x, edge_index, W1, b1, W2, b2)
    # the compiled stream layout depends on the graph: key the cache on it
    key = (meta["L"], meta["T"], tuple(meta["slices"]))
    if key not in _CACHE:
        nc = build_kernel(meta)
        _CACHE[key] = SpmdRunner(nc)
        _CACHE["k"] = _CACHE[key]  # test.py compatibility alias
    runner = _CACHE[key]
    results = runner(in_maps)
    lg, ls = postprocess(results)
    return lg.astype(np.float32), ls.astype(np.float32)



# revision 11
# speedup vs baseline: 1.6979x; 1.6979x over previous
"""2-layer GCN (GCNConv x2 + log_softmax) on 8 Trainium2 NeuronCores.

Sharding: destination-node rows are split across the 8 cores (S=12544 nodes
each, incl. 352 zero phantom pad nodes); W1/W2 replicated. Per layer the
scaled feature table g = deg^-1/2 * (h @ W) is all-gathered into every core
as a feature-transposed SBUF table [128, S] (partition 16*g+f = feature f of
source shard g). The per-edge gather runs on GPSIMD ap_gather (8 Q7 cores,
one per source shard). Segment sums use an exact-length-class edge layout
(host-sorted) + DVE strided reduces, then an ap_gather un-permute back to
node order and a PE matmul combine across the 8 source groups.

Perf structure: the table is appended to classout in one SBUF tile [128,
T+S], so degree-1-in-group dests skip the edge stream entirely (their
aggregate IS one table column; the un-permute fetches it directly). The
un-permute is sliced per 512-column post tile so the PE/DVE post phase
overlaps the Pool gather instead of serializing after it.
"""
import numpy as np

NC = 8            # cores
NG = 8            # source groups (= shards)
S = 12544         # nodes per shard
V = NC * S        # padded node count
N_REAL = 100000
F = 16            # hidden dim
C8 = 8            # padded class count
X = 512           # input dim
TILE = 512        # post-phase column tile
GATHER_SLICE = 8192
ALIGN = 32        # stream cut alignment (idx offsets must be 4B aligned)


# --------------------------------------------------------------- tile patch
def _install_tile_patch():
    """The Tile tail drain accumulates more sem waits than this compiler
    allows on one CTRL instruction; spread them over SP nops (1 wait each)."""
    import concourse.tile as tile
    import concourse.mybir as mybir
    from concourse.vector_clock import ScopedClock
    if getattr(tile.TileContext, "_drain_patch", False):
        return
    _MAX_WAITS = 1

    def _patched(self, tick_clock, wait_clock):
        nc = self.nc
        nops = [nc.sync.nop(nofuse=True) for _ in range(40)]
        drain_inst = nc.sync.drain()
        wait_clock.add_sem_waits(
            drain_inst.ins, ScopedClock({None: tick_clock.global_clock})
        )
        si = drain_inst.ins.sync_info
        if si is not None and si.on_wait and len(si.on_wait) > _MAX_WAITS:
            waits = list(si.on_wait)
            si.on_wait.clear()
            chunks = [waits[i:i + _MAX_WAITS] for i in range(0, len(waits), _MAX_WAITS)]
            si.on_wait.extend(chunks[-1])
            rest = chunks[:-1]
            assert len(rest) <= len(nops), f"too many wait chunks: {len(rest)}"
            for nop, chunk in zip(nops, rest):
                nsi = nop.ins.sync_info
                if nsi is None:
                    nop.ins.sync_info = mybir.SyncInfo(on_wait=list(chunk), on_update=[])
                else:
                    nsi.on_wait.extend(chunk)
        nc.all_engine_barrier()
        assert self.sems is not None
        popped = nc._tile_sem_poison_stack.pop()
        assert popped is self._sem_poison
        nc.clear_and_free_semaphores(list(self.sems.allocated().values()))
        nc.all_engine_barrier()

    tile.TileContext._drain_and_barrier = _patched
    tile.TileContext._drain_patch = True


# ---------------------------------------------------------------- host prep
def preprocess(edge_index):
    row = np.asarray(edge_index[0], dtype=np.int64)
    col = np.asarray(edge_index[1], dtype=np.int64)
    deg_full = (np.bincount(col, minlength=V) + 1).astype(np.int32)

    core_of = (col // S).astype(np.int32)
    per_core = []
    cnt_all = np.zeros((NC, NG, S), dtype=np.int64)
    for k in range(NC):
        m = core_of == k
        r, c = row[m], col[m]
        g = (r // S).astype(np.int64)
        sloc = (r - g * S).astype(np.int64)
        dloc = (c - k * S).astype(np.int64)
        cnt = np.bincount(g * S + dloc, minlength=NG * S).reshape(NG, S)
        cnt_all[k] = cnt
        per_core.append((g, sloc, dloc, cnt))

    cmax = int(cnt_all.max())
    ncls = np.zeros((NC, NG, cmax + 1), dtype=np.int64)
    for k in range(NC):
        for g in range(NG):
            ncls[k, g] = np.bincount(cnt_all[k, g], minlength=cmax + 1)
    n_glob = ncls.max(axis=(0, 1))
    classes = [c for c in range(2, cmax + 1) if n_glob[c] > 0]  # c==1 via unperm
    n_pad = {}
    for c in classes:
        n = int(n_glob[c])
        t = 0
        while ((n + t) * c) % ALIGN != 0:
            t += 1
        n_pad[c] = n + t
    stream_start, pos_start = {}, {}
    soff, poff = 0, 1          # output position 0 reserved as zero
    for c in classes:
        stream_start[c] = soff
        pos_start[c] = poff
        soff += n_pad[c] * c
        poff += n_pad[c]
    L, T = soff, poff
    assert L % ALIGN == 0 and T < 32768

    cuts = [0]
    for c in classes:
        step = c
        while step % ALIGN != 0:
            step += c
        rs, re = stream_start[c], stream_start[c] + n_pad[c] * c
        cur = cuts[-1]
        while True:
            nxt = min(cur + GATHER_SLICE, re)
            if nxt >= re:
                if re > cur:
                    cuts.append(re)
                break
            snapped = rs + ((nxt - rs) // step) * step
            if snapped <= cur:
                snapped = min(cur + step, re)
            cuts.append(snapped)
            cur = snapped
    assert cuts[-1] == L
    slices = list(zip(cuts[:-1], cuts[1:]))
    max_slice = max(b - a for a, b in slices)

    slice_ops = []
    for a, b in slices:
        ops = []
        for c in classes:
            rs, re = stream_start[c], stream_start[c] + n_pad[c] * c
            lo, hi = max(rs, a), min(re, b)
            if lo >= hi:
                continue
            ops.append((lo - a, c, (hi - lo) // c, pos_start[c] + (lo - rs) // c))
        slice_ops.append(ops)

    inputs = []
    for k in range(NC):
        g, sloc, dloc, cnt = per_core[k]
        idx_streams = np.zeros((NG, L), dtype=np.int16)
        pos_arr = np.zeros((NG, S), dtype=np.int16)
        for gg in range(NG):
            m = g == gg
            sl, dl = sloc[m], dloc[m]
            cc = cnt[gg, dl]
            sl_raw, cc_raw = sl, cc
            order = np.lexsort((sl, dl, cc))
            sl, cc = sl[order], cc[order]
            bnd = np.searchsorted(cc, np.arange(1, cmax + 2))
            for c in classes:
                lo, hi = bnd[c - 1], bnd[c]
                if hi > lo:
                    idx_streams[gg, stream_start[c]:stream_start[c] + (hi - lo)] = sl[lo:hi]
            arr = cnt[gg]
            src1 = np.zeros(S, dtype=np.int64)
            e1 = cc_raw == 1
            src1[dl[e1]] = sl_raw[e1]
            sorted_d = np.argsort(arr, kind="stable")
            arr_s = arr[sorted_d]
            first = np.searchsorted(arr_s, arr_s)
            rank = np.arange(S) - first
            ps = np.zeros(S, dtype=np.int64)
            starts = np.array([pos_start[int(c)] if c > 1 else 0 for c in arr_s])
            ps[sorted_d] = np.where(arr_s > 1, starts + rank, 0)
            ps = np.where(arr == 1, T + src1, ps)
            pos_arr[gg] = ps.astype(np.int16)

        def pack16(mat, width):
            out = np.zeros((128, width // 16), np.int16)
            for gg in range(NG):
                out[16 * gg:16 * gg + 16, :] = mat[gg].reshape(width // 16, 16).T
            return out

        inputs.append({
            "idx": pack16(idx_streams, L),
            "unperm": pack16(pos_arr, S),
            "deg": deg_full[k * S:(k + 1) * S].reshape(1, S),
        })
    meta = dict(L=L, T=T, slices=slices, slice_ops=slice_ops, max_slice=max_slice)
    return inputs, meta


# ---------------------------------------------------------------- kernel
def build_kernel(meta, reps=1):
    import concourse.bass as bass
    import concourse.mybir as mybir
    from concourse import bacc
    from concourse.tile import TileContext
    _install_tile_patch()
    AF = mybir.ActivationFunctionType
    DT = mybir.dt
    L, T = meta["L"], meta["T"]
    slices, slice_ops = meta["slices"], meta["slice_ops"]
    max_slice = meta["max_slice"]

    nc = bacc.Bacc(None, target_bir_lowering=False, num_devices=NC)
    f32 = DT.float32

    xt_d = nc.dram_tensor("xt", [X, S], f32, kind="ExternalInput")
    deg_d = nc.dram_tensor("deg", [1, S], DT.int32, kind="ExternalInput")
    idx_d = nc.dram_tensor("idx", [128, L // 16], DT.int16, kind="ExternalInput")
    unp_d = nc.dram_tensor("unperm", [128, S // 16], DT.int16, kind="ExternalInput")
    w1_d = nc.dram_tensor("W1", [X, F], f32, kind="ExternalInput")
    b1_d = nc.dram_tensor("b1", [F, 1], f32, kind="ExternalInput")
    w2_d = nc.dram_tensor("W2", [F, C8], f32, kind="ExternalInput")
    b2_d = nc.dram_tensor("b2", [C8, 1], f32, kind="ExternalInput")
    pcomb_d = nc.dram_tensor("pcomb", [128, F], f32, kind="ExternalInput")
    ones7_d = nc.dram_tensor("ones7", [C8, 1], f32, kind="ExternalInput")
    ones18_d = nc.dram_tensor("ones18", [1, C8], f32, kind="ExternalInput")
    lg_d = nc.dram_tensor("lg_t", [C8, S], f32, kind="ExternalOutput")
    ls_d = nc.dram_tensor("ls_t", [C8, S], f32, kind="ExternalOutput")

    def widths():
        off = 0
        while off < S:
            w = min(TILE, S - off)
            yield off, w
            off += w

    with TileContext(nc) as tc:
        with tc.tile_pool(name="dram", bufs=1, space="DRAM") as dram, \
             tc.tile_pool(name="const", bufs=1) as constp:
            idx_t = constp.tile([128, L // 16], DT.int16)
            nc.sync.dma_start(out=idx_t[:], in_=idx_d[:])
            unp_t = constp.tile([128, S // 16], DT.int16)
            nc.sync.dma_start(out=unp_t[:], in_=unp_d[:])
            w1_t = constp.tile([128, 4 * F], f32)
            for kc in range(4):
                nc.sync.dma_start(out=w1_t[:, kc * F:(kc + 1) * F],
                                  in_=w1_d[kc * 128:(kc + 1) * 128, :])
            w2_t = constp.tile([F, C8], f32)
            nc.sync.dma_start(out=w2_t[:], in_=w2_d[:])
            b1_t = constp.tile([F, 1], f32)
            nc.sync.dma_start(out=b1_t[:], in_=b1_d[:])
            b2_t = constp.tile([C8, 1], f32)
            nc.sync.dma_start(out=b2_t[:], in_=b2_d[:])
            pcomb_t = constp.tile([128, F], f32)
            nc.sync.dma_start(out=pcomb_t[:], in_=pcomb_d[:])
            ones7_t = constp.tile([C8, 1], f32)
            nc.sync.dma_start(out=ones7_t[:], in_=ones7_d[:])
            ones18_t = constp.tile([1, C8], f32)
            nc.sync.dma_start(out=ones18_t[:], in_=ones18_d[:])
            zero8_t = constp.tile([C8, TILE], f32)
            nc.gpsimd.memset(zero8_t[:], 0.0)

          # (reps>1 repeats the whole pipeline for HW timing)
            for _rep in range(reps):
                dinv_d = dram.tile([1, S], f32, tag=f"dinv{_rep}")
                cc_in1 = dram.tile([F, S], f32, tag=f"ci1_{_rep}")
                cc_out1 = dram.tile([128, S], f32, addr_space="Shared",
                                    tag=f"co1_{_rep}")
                cc_in2 = dram.tile([F, S], f32, tag=f"ci2_{_rep}")
                cc_out2 = dram.tile([128, S], f32, addr_space="Shared",
                                    tag=f"co2_{_rep}")
                with tc.tile_pool(name=f"p0_{_rep}", bufs=1) as p0:
                    dg = p0.tile([1, S], DT.int32)
                    nc.sync.dma_start(out=dg[:], in_=deg_d[:])
                    dgf = p0.tile([1, S], f32)
                    nc.vector.tensor_copy(out=dgf[:], in_=dg[:])
                    sq = p0.tile([1, S], f32)
                    nc.scalar.activation(out=sq[:], in_=dgf[:], func=AF.Sqrt)
                    dv = p0.tile([1, S], f32)
                    nc.vector.reciprocal(out=dv[:], in_=sq[:])
                    nc.sync.dma_start(out=dinv_d[:], in_=dv[:])

                with tc.tile_pool(name=f"mmx{_rep}", bufs=4) as mmx, \
                     tc.tile_pool(name=f"mmo{_rep}", bufs=3) as mmo, \
                     tc.tile_pool(name=f"ps1_{_rep}", bufs=3, space="PSUM") as ps1:
                    for off, w in widths():
                        psum = ps1.tile([F, TILE], f32, tag="ps")
                        for kc in range(4):
                            xt_t = mmx.tile([128, TILE], f32, tag="xt")
                            nc.sync.dma_start(
                                out=xt_t[:, :w],
                                in_=xt_d[kc * 128:(kc + 1) * 128, off:off + w])
                            nc.tensor.matmul(
                                out=psum[:, :w],
                                lhsT=w1_t[:, kc * F:(kc + 1) * F],
                                rhs=xt_t[:, :w],
                                start=(kc == 0), stop=(kc == 3))
                        dvr = mmo.tile([F, TILE], f32, tag="dvr")
                        nc.sync.dma_start(
                            out=dvr[:, :w],
                            in_=dinv_d[0:1, off:off + w].to_broadcast([F, w]))
                        g1 = mmo.tile([F, TILE], f32, tag="g1")
                        nc.vector.tensor_tensor(out=g1[:, :w], in0=psum[:, :w],
                                                in1=dvr[:, :w],
                                                op=mybir.AluOpType.mult)
                        nc.sync.dma_start(out=cc_in1[:, off:off + w], in_=g1[:, :w])
                        nc.sync.dma_start(out=cc_in2[C8:F, off:off + w],
                                          in_=zero8_t[:, :w])

                for layer in (1, 2):
                    cc_in = cc_in1 if layer == 1 else cc_in2
                    cc_out = cc_out1 if layer == 1 else cc_out2
                    nc.gpsimd.collective_compute(
                        "AllGather", mybir.AluOpType.bypass,
                        replica_groups=[list(range(NC))],
                        ins=[cc_in[:]], outs=[cc_out[:]])

                    with tc.tile_pool(name=f"cls{layer}_{_rep}", bufs=1) as clsp:
                        classout = clsp.tile([128, T + S], f32)
                        nc.gpsimd.memset(classout[:, 0:1], 0.0)
                        with tc.tile_pool(name=f"sl{layer}_{_rep}", bufs=2) as slp:
                            table = classout[:, T:T + S]
                            nc.sync.dma_start(out=table, in_=cc_out[:])
                            for (a, b), ops in zip(slices, slice_ops):
                                ln = b - a
                                sl = slp.tile([128, max_slice], f32, tag="sl")
                                nc.gpsimd.ap_gather(
                                    out_ap=sl[:, :ln].rearrange("p (n d) -> p n d", d=1),
                                    in_ap=table.rearrange("p (n d) -> p n d", d=1),
                                    idxs_ap=idx_t[:, a // 16:b // 16],
                                    channels=128, num_elems=S, d=1, num_idxs=ln)
                                for (loff, c, nseg, pos) in ops:
                                    if c == 1:
                                        nc.vector.tensor_copy(
                                            out=classout[:, pos:pos + nseg],
                                            in_=sl[:, loff:loff + nseg])
                                    else:
                                        nc.vector.tensor_reduce(
                                            out=classout[:, pos:pos + nseg],
                                            in_=sl[:, loff:loff + nseg * c]
                                                .rearrange("p (n c) -> p n c", c=c),
                                            axis=mybir.AxisListType.X,
                                            op=mybir.AluOpType.add)

                        with tc.tile_pool(name=f"al{layer}_{_rep}", bufs=3) as alp:
                            with tc.tile_pool(name=f"po{layer}_{_rep}", bufs=3) as po, \
                                 tc.tile_pool(name=f"pp{layer}_{_rep}", bufs=2,
                                              space="PSUM") as pp:
                                for off, w in widths():
                                    aligned = alp.tile([128, TILE], f32, tag="al")
                                    nc.gpsimd.ap_gather(
                                        out_ap=aligned[:, :w]
                                            .rearrange("p (n d) -> p n d", d=1),
                                        in_ap=classout[:]
                                            .rearrange("p (n d) -> p n d", d=1),
                                        idxs_ap=unp_t[:, off // 16:(off + w) // 16],
                                        channels=128, num_elems=T + S, d=1,
                                        num_idxs=w)
                                    agg = pp.tile([F, TILE], f32, tag="agg")
                                    nc.tensor.matmul(
                                        out=agg[:, :w], lhsT=pcomb_t[:],
                                        rhs=aligned[:, :w],
                                        start=True, stop=True)
                                    own = po.tile([F, TILE], f32, tag="own")
                                    nc.sync.dma_start(out=own[:, :w],
                                                      in_=cc_in[:, off:off + w])
                                    dvr = po.tile([F, TILE], f32, tag="dvr")
                                    nc.sync.dma_start(
                                        out=dvr[:, :w],
                                        in_=dinv_d[0:1, off:off + w].to_broadcast([F, w]))
                                    t0 = po.tile([F, TILE], f32, tag="t0")
                                    nc.vector.tensor_add(out=t0[:, :w], in0=agg[:, :w],
                                                         in1=own[:, :w])
                                    nc.vector.tensor_tensor(out=t0[:, :w], in0=t0[:, :w],
                                                            in1=dvr[:, :w],
                                                            op=mybir.AluOpType.mult)
                                    if layer == 1:
                                        h1 = po.tile([F, TILE], f32, tag="h1")
                                        nc.scalar.activation(out=h1[:, :w], in_=t0[:, :w],
                                                             func=AF.Relu,
                                                             bias=b1_t[:, 0:1])
                                        t2 = pp.tile([C8, TILE], f32, tag="t2")
                                        nc.tensor.matmul(out=t2[:, :w], lhsT=w2_t[:],
                                                         rhs=h1[:, :w],
                                                         start=True, stop=True)
                                        g2 = po.tile([C8, TILE], f32, tag="g2")
                                        nc.vector.tensor_tensor(out=g2[:, :w],
                                                                in0=t2[:, :w],
                                                                in1=dvr[:C8, :w],
                                                                op=mybir.AluOpType.mult)
                                        nc.sync.dma_start(out=cc_in2[:C8, off:off + w],
                                                          in_=g2[:, :w])
                                    else:
                                        lg = po.tile([C8, TILE], f32, tag="lg")
                                        nc.scalar.activation(out=lg[:, :w],
                                                             in_=t0[:C8, :w],
                                                             func=AF.Identity,
                                                             bias=b2_t[:, 0:1])
                                        nc.sync.dma_start(out=lg_d[:, off:off + w],
                                                          in_=lg[:, :w])
                                        ex = po.tile([C8, TILE], f32, tag="ex")
                                        nc.scalar.activation(out=ex[:, :w],
                                                             in_=lg[:, :w], func=AF.Exp)
                                        sm = pp.tile([1, TILE], f32, tag="sm")
                                        nc.tensor.matmul(out=sm[:, :w], lhsT=ones7_t[:],
                                                         rhs=ex[:, :w],
                                                         start=True, stop=True)
                                        lsm = po.tile([1, TILE], f32, tag="lsm")
                                        nc.scalar.activation(out=lsm[:, :w],
                                                             in_=sm[:, :w], func=AF.Ln)
                                        lsb = pp.tile([C8, TILE], f32, tag="lsb")
                                        nc.tensor.matmul(out=lsb[:, :w],
                                                         lhsT=ones18_t[:],
                                                         rhs=lsm[:, :w],
                                                         start=True, stop=True)
                                        ls = po.tile([C8, TILE], f32, tag="ls")
                                        nc.vector.tensor_tensor(
                                            out=ls[:, :w], in0=lg[:, :w],
                                            in1=lsb[:, :w],
                                            op=mybir.AluOpType.subtract)
                                        nc.sync.dma_start(out=ls_d[:, off:off + w],
                                                          in_=ls[:, :w])
    nc.compile()
    return nc


def make_const_inputs(W1, b1, W2, b2):
    pcomb = np.zeros((128, F), np.float32)
    for g in range(NG):
        for f in range(F):
            pcomb[16 * g + f, f] = 1.0
    ones7 = np.zeros((C8, 1), np.float32); ones7[:7, 0] = 1.0
    ones18 = np.ones((1, C8), np.float32)
    w2p = np.zeros((F, C8), np.float32); w2p[:, :7] = np.asarray(W2, np.float32)
    b2p = np.zeros((C8, 1), np.float32); b2p[:7, 0] = np.asarray(b2, np.float32)
    return {
        "W1": np.asarray(W1, np.float32),
        "b1": np.asarray(b1, np.float32).reshape(F, 1),
        "W2": w2p, "b2": b2p,
        "pcomb": pcomb, "ones7": ones7, "ones18": ones18,
    }


def prepare_all(x, edge_index, W1, b1, W2, b2):
    per_core, meta = preprocess(edge_index)
    consts = make_const_inputs(W1, b1, W2, b2)
    xt = np.zeros((X, V), np.float32)
    xt[:, :N_REAL] = np.asarray(x, np.float32).T
    in_maps = []
    for k in range(NC):
        m = dict(per_core[k])
        m.update(consts)
        m["xt"] = np.ascontiguousarray(xt[:, k * S:(k + 1) * S])
        in_maps.append(m)
    return in_maps, meta


# ---------------------------------------------------------------- runner
class SpmdRunner:
    def __init__(self, nc, n_cores=NC):
        import jax
        import concourse.mybir as mybir
        from concourse import bass2jax
        from jax.sharding import Mesh, PartitionSpec
        from jax.experimental.shard_map import shard_map
        bass2jax.install_neuronx_cc_hook()
        self.jax = jax
        self.n_cores = n_cores
        partition_name = nc.partition_id_tensor.name if nc.partition_id_tensor else None
        in_names, out_names, out_avals, zero_outs = [], [], [], []
        for alloc in nc.m.functions[0].allocations:
            if not isinstance(alloc, mybir.MemoryLocationSet):
                continue
            name = alloc.memorylocations[0].name
            if alloc.kind == "ExternalInput":
                if name != partition_name:
                    in_names.append(name)
            elif alloc.kind == "ExternalOutput":
                shape = tuple(alloc.tensor_shape)
                dtype = mybir.dt.np(alloc.dtype)
                out_names.append(name)
                out_avals.append(jax.core.ShapedArray(shape, dtype))
                zero_outs.append(np.zeros(shape, dtype))
        self.in_names, self.out_names = in_names, out_names
        self.out_avals, self.zero_outs = out_avals, zero_outs
        n_params, n_outs = len(in_names), len(out_names)
        all_in_names = list(in_names) + list(out_names)
        if partition_name is not None:
            all_in_names.append(partition_name)

        def _body(*args):
            operands = list(args)
            if partition_name is not None:
                operands.append(bass2jax.partition_id_tensor())
            outs = bass2jax._bass_exec_p.bind(
                *operands,
                out_avals=tuple(out_avals),
                in_names=tuple(all_in_names),
                out_names=tuple(out_names),
                lowering_input_output_aliases=(),
                sim_require_finite=False,
                sim_require_nnan=False,
                nc=nc,
            )
            return tuple(outs)

        devices = jax.devices()[:n_cores]
        self.mesh = Mesh(np.asarray(devices), ("core",))
        in_specs = (PartitionSpec("core"),) * (n_params + n_outs)
        out_specs = (PartitionSpec("core"),) * n_outs
        self.fn = jax.jit(
            shard_map(_body, mesh=self.mesh, in_specs=in_specs,
                      out_specs=out_specs, check_rep=False),
            keep_unused=True,
        )

    def _concat(self, in_maps):
        n = self.n_cores
        per_core = [[np.asarray(m[name]) for name in self.in_names] for m in in_maps]
        concat_in = [np.concatenate([per_core[c][i] for c in range(n)], axis=0)
                     for i in range(len(self.in_names))]
        concat_zeros = [np.zeros((n * z.shape[0], *z.shape[1:]), z.dtype)
                        for z in self.zero_outs]
        return concat_in + concat_zeros

    def __call__(self, in_maps):
        jax = self.jax
        out = self.fn(*self._concat(in_maps))
        jax.block_until_ready(out)
        n = self.n_cores
        return [
            {name: np.asarray(out[i]).reshape(n, *self.out_avals[i].shape)[c]
             for i, name in enumerate(self.out_names)}
            for c in range(n)
        ]

    def time_it(self, in_maps, reps=5):
        import time
        jax = self.jax
        from jax.sharding import NamedSharding, PartitionSpec
        sh = NamedSharding(self.mesh, PartitionSpec("core"))
        args = [jax.device_put(a, sh) for a in self._concat(in_maps)]
        out = self.fn(*args); jax.block_until_ready(out)
        ts = []
        for _ in range(reps):
            t0 = time.perf_counter()
            out = self.fn(*args)
            jax.block_until_ready(out)
            ts.append(time.perf_counter() - t0)
        return min(ts), ts


def postprocess(results):
    lg = np.concatenate([r["lg_t"].T for r in results], axis=0)[:N_REAL, :7]
    ls = np.concatenate([r["ls_t"].T for r in results], axis=0)[:N_REAL, :7]
    return lg, ls


_CACHE = {}


def kernel(x, edge_index, W1, b1, W2, b2):
    in_maps, meta = prepare_all(x, edge_index, W1, b1, W2, b2)
    # the compiled stream layout depends on the graph: key the cache on it
    key = (meta["L"], meta["T"], tuple(meta["slices"]))
    if key not in _CACHE:
        nc = build_kernel(meta)
        _CACHE[key] = SpmdRunner(nc)
        _CACHE["k"] = _CACHE[key]  # test.py compatibility alias
    runner = _CACHE[key]
    results = runner(in_maps)
    lg, ls = postprocess(results)
    return lg.astype(np.float32), ls.astype(np.float32)

